# revision 34
# baseline (speedup 1.0000x reference)
"""Trainium2 Bass kernel for the tiny EEG CNN (nn_CNN_56745107915038).

Single-core latency-bound graph (~2.8 MFLOP), replicated SPMD on all 8
cores; core 0's output is returned. v3 design:

  - kernel() packs x-views and ALL weights into one [64, F] f32 tensor on
    the host (pure data marshaling: transposes/duplication, no math), plus
    a bf16 copy of the eeg block for the conv moving operand. The device
    program does 3 input DMAs total (pack cols 0:384 with the x-derived
    data first, the weight remainder second, xbf third), so every weight
    is on-chip by ~3.5us and no engine ever waits on a DMA slot chain.
  - e0 is packed twice so dots = one tensor_tensor_reduce against the
    [wav_a; wav_b] pair - no PE broadcast matmul.
  - se_w1.T / se_w2.T / conv_w.T / fcn_w1.T are packed pre-transposed:
    no PE transpose + PSUM->SBUF copy chains anywhere.
  - diag(t) via one two-scalar tensor_scalar (I2*dots/sab); softmax of the
    SE weights deferred: exp-scaled conv stationary (split DVE/ACT), 1/sum
    rides the Relu scale; 2-class softmax = sigmoid of PM-folded logits
    with fcn_b2's contribution as a second accumulating matmul.
  - PE p-state warmup; TileContext teardown stripped (NEFF runs once per
    PJRT dispatch).
"""

import sys

for _p in ("/opt/trn_rl_repo", "/root/.axon_site/_ro/trn_rl_repo"):
    if _p not in sys.path:
        sys.path.append(_p)

import numpy as np

from concourse import bass, mybir
from concourse import tile
from concourse.bass_utils import run_bass_kernel_spmd
from concourse.tile_rust import add_dep_helper

F32 = mybir.dt.float32
BF16 = mybir.dt.bfloat16
ALU = mybir.AluOpType
ACTF = mybir.ActivationFunctionType

N_CORES = 8
KW = 9
WOUT = 128 - KW + 1  # 120

# ---- bf16 x-pack column layout ----
_E0 = 0        # eeg [64,128]
_WAB = 128     # [wav_a; wav_b] rows 0-1, cols 128:256
_E0X2 = 256    # [e0; e0] rows 0-1, cols 256:384
_FB = 384
# ---- f32 weight-pack column layout ----
_W1T = 0       # se_w1.T [64,64]
_W2T = 64      # se_w2.T [64,64]
_B1SE = 128    # se_b1 col
_B2SE = 129    # se_b2 col
_CWT = 130     # conv_w.T [64, 9, 10] (r, k, o)
_W1P = 220     # fcn_w1.T [20,10] rows 0-19
_B1C = 230     # fcn_b1 rows 0-9
_W2W = 231     # fcn_w2 [2,10] rows 0-1
_B2C = 241     # fcn_b2 rows 0-1
_CB = 242      # conv_b rows 0-9
_PM = 243      # [[1,-1],[-1,1]] rows 0-1, cols 243:245
_MASK2 = 245   # MASK2[g, o*2+g'] = (g==g') rows 0-1, cols 245:265
_MASKO = 265   # MASKO[o, o'*2+g] = (o==o') rows 0-9, cols 265:285
_I2 = 285      # identity [2,2] rows 0-1, cols 285:287
_F = 287


def _split_multi_waits(nc):
    """Walrus in this container allows at most one sync wait per instruction.

    Tile's sem assignment freely attaches several. Hoist all but the last
    wait of each instruction onto injected same-engine NOPs placed directly
    before it -- engines execute in order, so the waits still gate it.
    """
    for fn in nc.m.functions:
        for blk in fn.blocks:
            new = []
            for inst in blk.instructions:
                si = inst.sync_info
                if si is not None and len(si.on_wait) > 1:
                    waits = sorted(
                        si.on_wait, key=lambda w: 0 if "DMA" in (w.ant_name or "") else 1
                    )
                    for j, w in enumerate(waits[:-1]):
                        new.append(
                            mybir.InstNoOp(
                                name=f"{inst.name}-swait{j}",
                                engine=inst.engine,
                                ins=[], outs=[],
                                sync_info=mybir.SyncInfo(on_wait=[w], on_update=[]),
                            )
                        )
                    inst.sync_info = mybir.SyncInfo(
                        on_wait=[waits[-1]], on_update=list(si.on_update)
                    )
                new.append(inst)
            blk.instructions = new


class _TileContext(tile.TileContext):
    """TileContext with an empty kernel tail.

    The NEFF runs once per PJRT dispatch, so semaphores never need
    resetting and the runtime's own DMA-queue quiescence covers the
    output DMA completion. Skip the drain/barrier/sem-clear sequence.
    """

    def _drain_and_barrier(self, tick_clock, wait_clock):
        popped = self.nc._tile_sem_poison_stack.pop()
        assert popped is self._sem_poison


def _strip_out_dma_sem(nc):
    """Drop the completion-sem update from the final output DMA.

    Nothing waits on it (the teardown is empty; the runtime's DMA-queue
    quiescence covers completion), and the cost model charges 900ns of
    sem propagation only when a DMA carries updates."""
    last_dma = None
    for blk in nc.m.functions[0].blocks:
        for inst in blk.instructions:
            if type(inst).__name__ == "InstDMACopy":
                last_dma = inst
    if last_dma is not None and last_dma.sync_info is not None:
        last_dma.sync_info = mybir.SyncInfo(
            on_wait=list(last_dma.sync_info.on_wait), on_update=[]
        )


def _strip_preamble_barrier(nc):
    """Drop the const-init all-engine barrier from the Bass preamble."""
    blk0 = nc.m.functions[0].blocks[0]
    keep = [
        i for i in blk0.instructions
        if type(i).__name__ not in ("InstDrain", "InstEventSemaphore")
    ]
    blk0.instructions = keep


def build_program(split_waits=True):
    nc = bass.Bass()

    packbf = nc.dram_tensor("packbf", [64, _FB], BF16, kind="ExternalInput")
    packed = nc.dram_tensor("packed", [64, _F], F32, kind="ExternalInput")
    out = nc.dram_tensor("out", [1, 2], F32, kind="ExternalOutput")

    with _TileContext(nc) as tc:
        with (
            tc.tile_pool(name="sb", bufs=1) as sb,
            tc.tile_pool(name="ps", bufs=1, space="PSUM") as ps,
        ):
            # ---------------- SBUF tiles ----------------
            P = sb.tile([64, _F], F32, tag="P")          # weight pack
            PB = sb.tile([64, _FB], BF16, tag="PB")      # x pack
            ones64 = sb.tile([64, 64], F32, tag="ones64")
            stall = sb.tile([64, 9, 20], BF16, tag="stall")
            junkA = sb.tile([2, 128], BF16, tag="junkA")
            junkB = sb.tile([2, 128], BF16, tag="junkB")
            Esq = sb.tile([64, 128], BF16, tag="Esq")
            ssq2 = sb.tile([64, 2], F32, tag="ssq2")     # [ssq_e | ssqab]
            rt2 = sb.tile([64, 2], F32, tag="rt2")       # sqrt of ssq2
            rq2 = sb.tile([64, 2], F32, tag="rq2")
            dots = sb.tile([2, 1], F32, tag="dots")
            D2 = sb.tile([2, 2], F32, tag="D2")
            v_sb = sb.tile([64, 1], F32, tag="v_sb")
            hT = sb.tile([64, 2], F32, tag="hT")
            expT = sb.tile([64, 2], F32, tag="expT")
            rs = sb.tile([2, 1], F32, tag="rs")
            scol = sb.tile([20, 1], F32, tag="scol")
            bcol = sb.tile([20, 1], F32, tag="bcol")
            W2pm = sb.tile([10, 2], F32, tag="W2pm")
            R = sb.tile([20, 120], F32, tag="R")
            msum = sb.tile([20, 1], F32, tag="msum")
            h2 = sb.tile([10, 1], F32, tag="h2")
            final = sb.tile([1, 2], F32, tag="final")

            # pack views
            E = PB[:, _E0:_E0 + 128]
            Wab = PB[0:2, _WAB:_WAB + 128]
            E0x2 = PB[0:2, _E0X2:_E0X2 + 128]
            w1T = P[:, _W1T:_W1T + 64]
            w2T = P[:, _W2T:_W2T + 64]
            b1se = P[:, _B1SE:_B1SE + 1]
            b2se = P[:, _B2SE:_B2SE + 1]
            CWT = P[:, _CWT:_CWT + 90]
            W1p = P[0:20, _W1P:_W1P + 10]
            b1col = P[0:10, _B1C:_B1C + 1]
            W2w = P[0:2, _W2W:_W2W + 10]
            b2col = P[0:2, _B2C:_B2C + 1]
            cb10 = P[0:10, _CB:_CB + 1]
            PM = P[0:2, _PM:_PM + 2]
            MASK2 = P[0:2, _MASK2:_MASK2 + 20]
            MASKO = P[0:10, _MASKO:_MASKO + 20]
            I2 = P[0:2, _I2:_I2 + 2]

            # ---------------- PSUM tiles ----------------
            junk_ps = ps.tile([2, 8], F32, tag="pE")
            v_ps = ps.tile([64, 1], F32, tag="pB")
            tbc_ps = ps.tile([64, 2], F32, tag="pC")
            bcol_ps = ps.tile([20, 1], F32, tag="pG")
            w2pm_ps = ps.tile([10, 2], F32, tag="pF")

            # ---------------- DMAs (all on SP) ----------------
            nc.sync.dma_start(out=PB[:], in_=packbf[:, :])
            nc.sync.dma_start(out=P[:], in_=packed[:, :])

            # ---------------- constants + PE warmup ----------------
            nc.vector.memset(ones64[:], 1.0)
            for _ in range(2):
                nc.tensor.matmul(
                    junk_ps[0:1, 0:1], ones64[0:1, 0:1], ones64[0:1, 0:1],
                    start=True, stop=True,
                )

            # early, DMA-ready PE work: bcol and W2pm
            bcol_i = nc.tensor.matmul(bcol_ps[:], MASKO, cb10, start=True, stop=True)
            bcolcp_i = nc.vector.tensor_copy(bcol[:], bcol_ps[:])
            w2pm_i = nc.tensor.matmul(w2pm_ps[:], W2w, PM, start=True, stop=True)
            w2pmcp_i = nc.vector.tensor_copy(W2pm[:], w2pm_ps[:])

            # ---------------- cosine stage ----------------
            # all reductions on DVE (accum_out is free there; ACT charges
            # +187ns per accumulator read). 1/sqrt(x) as sqrt(1/x): the
            # reciprocal runs BEFORE the one ACT Sqrt, so rt2 holds
            # [1/ne | 1/sab] and feeds v / D2 directly.
            # ssq_e on ACT (Square+accum); wav pair on DVE via
            # tensor_tensor + tensor_reduce (walrus here lacks
            # TensorTensorReduce: "ISA wrong length")
            nc.scalar.activation(
                Esq[:], E, ACTF.Square, accum_out=ssq2[:, 0:1]
            )
            dots_i = nc.vector.tensor_tensor(
                junkB[:], E0x2, Wab, op=ALU.mult
            )
            trd_i = nc.vector.tensor_reduce(
                dots[:], junkB[:], axis=mybir.AxisListType.X, op=ALU.add
            )
            ttr_ab = nc.gpsimd.tensor_tensor(
                junkA[:], Wab, Wab, op=ALU.mult
            )
            tra_i = nc.vector.tensor_reduce(
                ssq2[0:2, 1:2], junkA[:], axis=mybir.AxisListType.X, op=ALU.add
            )
            recb_i = nc.vector.reciprocal(rq2[0:2, 1:2], ssq2[0:2, 1:2])
            rece_i = nc.vector.reciprocal(rq2[:, 0:1], ssq2[:, 0:1])
            nc.scalar.activation(rt2[0:2, 1:2], rq2[0:2, 1:2], ACTF.Sqrt)
            nc.scalar.activation(rt2[:, 0:1], rq2[:, 0:1], ACTF.Sqrt)
            # D2 = diag(t) = (I2 * dots) * (1/sab)
            d2_i = nc.vector.tensor_scalar(
                out=D2[:], in0=I2, scalar1=dots[:], scalar2=rt2[0:2, 1:2],
                op0=ALU.mult, op1=ALU.mult,
            )
            d2a_i = d2_i

            # ---------------- SE chain ----------------
            v_i = nc.tensor.matmul(v_ps[:], w1T, rt2[:, 0:1], start=True, stop=True)
            tbc_i = nc.tensor.matmul(
                tbc_ps[:], ones64[0:2, :], D2[:], start=True, stop=True
            )
            vcp_i = nc.vector.tensor_copy(v_sb[:], v_ps[:])
            nc.scalar.activation(
                hT[:], tbc_ps[:], ACTF.Tanh, bias=b1se, scale=v_sb[:]
            )
            z_ps = ps.tile([64, 2], F32, tag="pD")
            z_i = nc.tensor.matmul(z_ps[:], w2T, hT[:], start=True, stop=True)
            sT_ps = ps.tile([64, 2], F32, tag="pF")
            nc.scalar.activation(sT_ps[:], z_ps[:], ACTF.Sigmoid, bias=b2se)
            nc.scalar.activation(expT[:], sT_ps[:], ACTF.Exp)

            # stall[r,k,o*2+g]: both halves on DVE (107ns each at bf16
            # 2x rate; beats ACT's 260ns Copy)
            stall1_i = nc.vector.tensor_scalar_mul(
                stall[:, :, 0:20:2], CWT, expT[:, 0:1]
            )
            stall2_i = nc.vector.tensor_scalar_mul(
                stall[:, :, 1:20:2], CWT, expT[:, 1:2]
            )

            # softmax denominators
            cs_ps = ps.tile([2, 1], F32, tag="pB")
            cs_i = nc.tensor.matmul(
                cs_ps[:], expT[:], ones64[:, 0:1], start=True, stop=True
            )
            rs_i = nc.vector.reciprocal(rs[:], cs_ps[:])
            scol_ps = ps.tile([20, 1], F32, tag="pC")
            scol_i = nc.tensor.matmul(scol_ps[:], MASK2, rs[:], start=True, stop=True)
            scolcp_i = nc.vector.tensor_copy(scol[:], scol_ps[:])

            # ---------------- conv: 9 accumulated matmuls ----------------
            Y_ps = ps.tile([20, 120], F32, tag="pA")
            conv_is = []
            for k in range(KW):
                conv_is.append(nc.tensor.matmul(
                    Y_ps[:],
                    stall[:, k, :],
                    E[:, k:k + WOUT],
                    start=(k == 0), stop=(k == KW - 1),
                ))

            # relu(Y/colsum + b) with mean via ACT accum (the +187ns
            # accumulator read still beats a DVE round trip here)
            nc.scalar.activation(
                R[:], Y_ps[:], ACTF.Relu, bias=bcol[:], scale=scol[:],
                accum_out=msum[:],
            )

            # ---------------- fcn head ----------------
            S_ps = ps.tile([10, 1], F32, tag="pB")
            s_i = nc.tensor.matmul(S_ps[:], W1p, msum[:], start=True, stop=True)
            nc.scalar.activation(
                h2[:], S_ps[:], ACTF.Sigmoid, bias=b1col, scale=1.0 / WOUT
            )
            logit_ps = ps.tile([1, 2], F32, tag="pC")
            lg2_i = nc.tensor.matmul(logit_ps[:], b2col, PM, start=True, stop=False)
            lg1_i = nc.tensor.matmul(logit_ps[:], h2[:], W2pm[:], start=False, stop=True)
            nc.scalar.activation(final[:], logit_ps[:], ACTF.Sigmoid)

            nc.sync.dma_start(out=out[:, :], in_=final[:])

            # ---- queue-order pins (scheduler-only edges, no sems) ----
            pe_order = [bcol_i, w2pm_i, v_i, tbc_i, z_i, cs_i, scol_i,
                        conv_is[0], conv_is[-1], s_i, lg2_i, lg1_i]
            for a, b in zip(pe_order[1:], pe_order[:-1]):
                add_dep_helper(a.ins, b.ins, sync=False, reason="pe order")
            dve_order = [dots_i, trd_i, tra_i, recb_i, rece_i,
                         d2_i, vcp_i, bcolcp_i, w2pmcp_i, stall1_i,
                         stall2_i, rs_i, scolcp_i]
            for a, b in zip(dve_order[1:], dve_order[:-1]):
                add_dep_helper(a.ins, b.ins, sync=False, reason="dve order")

    _strip_preamble_barrier(nc)
    if split_waits:
        _split_multi_waits(nc)
    return nc


def _pack_inputs(inputs):
    import ml_dtypes
    f = {k: np.asarray(v, dtype=np.float32) for k, v in inputs.items()}
    x = f["x"].reshape(66, 128)
    eeg = x[1:65]                       # [64,128]
    pb = np.zeros((64, _FB), np.float32)
    pb[:, _E0:_E0 + 128] = eeg
    pb[0, _WAB:_WAB + 128] = x[0]
    pb[1, _WAB:_WAB + 128] = x[65]
    pb[0:2, _E0X2:_E0X2 + 128] = eeg[0]
    pk = np.zeros((64, _F), np.float32)
    pk[:, _W1T:_W1T + 64] = f["se_w1"].T
    pk[:, _W2T:_W2T + 64] = f["se_w2"].T
    pk[:, _B1SE] = f["se_b1"]
    pk[:, _B2SE] = f["se_b2"]
    # conv_w [10,1,64,9] -> [r, k, o]
    pk[:, _CWT:_CWT + 90] = np.transpose(
        f["conv_w"][:, 0, :, :], (1, 2, 0)
    ).reshape(64, 90)
    pk[0:20, _W1P:_W1P + 10] = f["fcn_w1"].T      # rows p=o*2+g
    pk[0:10, _B1C] = f["fcn_b1"]
    pk[0:2, _W2W:_W2W + 10] = f["fcn_w2"]
    pk[0:2, _B2C] = f["fcn_b2"]
    pk[0:10, _CB] = f["conv_b"]
    pk[0:2, _PM:_PM + 2] = np.array([[1.0, -1.0], [-1.0, 1.0]], np.float32)
    pk[0:2, _I2:_I2 + 2] = np.eye(2, dtype=np.float32)
    for o in range(10):
        for g in range(2):
            pk[g, _MASK2 + o * 2 + g] = 1.0
            pk[o, _MASKO + o * 2 + g] = 1.0
    return {"packbf": pb.astype(ml_dtypes.bfloat16), "packed": pk}


_NC_CACHE = None


def kernel(**inputs) -> np.ndarray:
    global _NC_CACHE
    if _NC_CACHE is None:
        _NC_CACHE = build_program()
    nc = _NC_CACHE

    in_map = _pack_inputs(inputs)
    res = run_bass_kernel_spmd(
        nc, [in_map] * N_CORES, core_ids=list(range(N_CORES))
    )
    return np.asarray(res.results[0]["out"], dtype=np.float32)


# revision 42
# speedup vs baseline: 1.1377x; 1.1377x over previous
"""Trainium2 Bass kernel for the tiny EEG CNN (nn_CNN_56745107915038).

Single-core latency-bound graph (~2.8 MFLOP), replicated SPMD on all 8
cores; core 0's output is returned. v3 design:

  - kernel() packs x-views and ALL weights into one [64, F] f32 tensor on
    the host (pure data marshaling: transposes/duplication, no math), plus
    a bf16 copy of the eeg block for the conv moving operand. The device
    program does 3 input DMAs total (pack cols 0:384 with the x-derived
    data first, the weight remainder second, xbf third), so every weight
    is on-chip by ~3.5us and no engine ever waits on a DMA slot chain.
  - e0 is packed twice so dots = one tensor_tensor_reduce against the
    [wav_a; wav_b] pair - no PE broadcast matmul.
  - se_w1.T / se_w2.T / conv_w.T / fcn_w1.T are packed pre-transposed:
    no PE transpose + PSUM->SBUF copy chains anywhere.
  - diag(t) via one two-scalar tensor_scalar (I2*dots/sab); softmax of the
    SE weights deferred: exp-scaled conv stationary (split DVE/ACT), 1/sum
    rides the Relu scale; 2-class softmax = sigmoid of PM-folded logits
    with fcn_b2's contribution as a second accumulating matmul.
  - PE p-state warmup; TileContext teardown stripped (NEFF runs once per
    PJRT dispatch).
"""

import sys

for _p in ("/opt/trn_rl_repo", "/root/.axon_site/_ro/trn_rl_repo"):
    if _p not in sys.path:
        sys.path.append(_p)

import numpy as np

from concourse import bass, mybir
from concourse import tile
from concourse.bass_utils import run_bass_kernel_spmd
from concourse.tile_rust import add_dep_helper

F32 = mybir.dt.float32
BF16 = mybir.dt.bfloat16
ALU = mybir.AluOpType
ACTF = mybir.ActivationFunctionType

N_CORES = 8
KW = 9
WOUT = 128 - KW + 1  # 120

# ---- bf16 x-pack column layout ----
_E0 = 0        # eeg [64,128]
_WAB = 128     # [wav_a; wav_b] rows 0-1, cols 128:256
_E0X2 = 256    # [e0; e0] rows 0-1, cols 256:384
_FB = 384
# ---- f32 weight-pack column layout ----
_W1T = 0       # se_w1.T [64,64]
_W2T = 64      # se_w2.T [64,64]
_B1SE = 128    # se_b1 col
_B2SE = 129    # se_b2 col
_CWT = 130     # conv_w.T [64, 9, 10] (r, k, o)
_W1P = 220     # fcn_w1.T [20,10] rows 0-19
_B1C = 230     # fcn_b1 rows 0-9
_W2W = 231     # fcn_w2 [2,10] rows 0-1
_B2C = 241     # fcn_b2 rows 0-1
_CB = 242      # conv_b rows 0-9
_PM = 243      # [[1,-1],[-1,1]] rows 0-1, cols 243:245
_MASK2 = 245   # MASK2[g, o*2+g'] = (g==g') rows 0-1, cols 245:265
_MASKO = 265   # MASKO[o, o'*2+g] = (o==o') rows 0-9, cols 265:285
_I2 = 285      # identity [2,2] rows 0-1, cols 285:287
_F = 287


def _split_multi_waits(nc):
    """Walrus in this container allows at most one sync wait per instruction.

    Tile's sem assignment freely attaches several. Hoist all but the last
    wait of each instruction onto injected same-engine NOPs placed directly
    before it -- engines execute in order, so the waits still gate it.
    """
    for fn in nc.m.functions:
        for blk in fn.blocks:
            new = []
            for inst in blk.instructions:
                si = inst.sync_info
                if si is not None and len(si.on_wait) > 1:
                    waits = sorted(
                        si.on_wait, key=lambda w: 0 if "DMA" in (w.ant_name or "") else 1
                    )
                    for j, w in enumerate(waits[:-1]):
                        new.append(
                            mybir.InstNoOp(
                                name=f"{inst.name}-swait{j}",
                                engine=inst.engine,
                                ins=[], outs=[],
                                sync_info=mybir.SyncInfo(on_wait=[w], on_update=[]),
                            )
                        )
                    inst.sync_info = mybir.SyncInfo(
                        on_wait=[waits[-1]], on_update=list(si.on_update)
                    )
                new.append(inst)
            blk.instructions = new


class _TileContext(tile.TileContext):
    """TileContext with an empty kernel tail.

    The NEFF runs once per PJRT dispatch, so semaphores never need
    resetting and the runtime's own DMA-queue quiescence covers the
    output DMA completion. Skip the drain/barrier/sem-clear sequence.
    """

    def _drain_and_barrier(self, tick_clock, wait_clock):
        popped = self.nc._tile_sem_poison_stack.pop()
        assert popped is self._sem_poison


def _strip_out_dma_sem(nc):
    """Drop the completion-sem update from the final output DMA.

    Nothing waits on it (the teardown is empty; the runtime's DMA-queue
    quiescence covers completion), and the cost model charges 900ns of
    sem propagation only when a DMA carries updates."""
    last_dma = None
    for blk in nc.m.functions[0].blocks:
        for inst in blk.instructions:
            if type(inst).__name__ == "InstDMACopy":
                last_dma = inst
    if last_dma is not None and last_dma.sync_info is not None:
        last_dma.sync_info = mybir.SyncInfo(
            on_wait=list(last_dma.sync_info.on_wait), on_update=[]
        )


def _strip_preamble_barrier(nc):
    """Drop the const-init all-engine barrier from the Bass preamble."""
    blk0 = nc.m.functions[0].blocks[0]
    keep = [
        i for i in blk0.instructions
        if type(i).__name__ not in ("InstDrain", "InstEventSemaphore")
    ]
    blk0.instructions = keep


def build_program(split_waits=True):
    nc = bass.Bass()

    packbf = nc.dram_tensor("packbf", [64, _FB], BF16, kind="ExternalInput")
    packed = nc.dram_tensor("packed", [64, _F], F32, kind="ExternalInput")
    out = nc.dram_tensor("out", [1, 2], F32, kind="ExternalOutput")

    with _TileContext(nc) as tc:
        with (
            tc.tile_pool(name="sb", bufs=1) as sb,
            tc.tile_pool(name="ps", bufs=1, space="PSUM") as ps,
        ):
            # ---------------- SBUF tiles ----------------
            P = sb.tile([64, _F], F32, tag="P")          # weight pack
            PB = sb.tile([64, _FB], BF16, tag="PB")      # x pack
            ones64 = sb.tile([64, 64], F32, tag="ones64")
            stall = sb.tile([64, 9, 20], BF16, tag="stall")
            junkA = sb.tile([2, 128], BF16, tag="junkA")
            junkB = sb.tile([2, 128], BF16, tag="junkB")
            Esq = sb.tile([64, 128], BF16, tag="Esq")
            ssq2 = sb.tile([64, 2], F32, tag="ssq2")     # [ssq_e | ssqab]
            rt2 = sb.tile([64, 2], F32, tag="rt2")       # sqrt of ssq2
            rq2 = sb.tile([64, 2], F32, tag="rq2")
            dots = sb.tile([2, 1], F32, tag="dots")
            D2 = sb.tile([2, 2], F32, tag="D2")
            v_sb = sb.tile([64, 1], F32, tag="v_sb")
            hT = sb.tile([64, 2], F32, tag="hT")
            expT = sb.tile([64, 2], F32, tag="expT")
            rs = sb.tile([2, 1], F32, tag="rs")
            scol = sb.tile([20, 1], F32, tag="scol")
            bcol = sb.tile([20, 1], F32, tag="bcol")
            W2pm = sb.tile([10, 2], F32, tag="W2pm")
            R = sb.tile([20, 120], F32, tag="R")
            msum = sb.tile([20, 1], F32, tag="msum")
            h2 = sb.tile([10, 1], F32, tag="h2")
            final = sb.tile([1, 2], F32, tag="final")

            # pack views
            E = PB[:, _E0:_E0 + 128]
            Wab = PB[0:2, _WAB:_WAB + 128]
            E0x2 = PB[0:2, _E0X2:_E0X2 + 128]
            w1T = P[:, _W1T:_W1T + 64]
            w2T = P[:, _W2T:_W2T + 64]
            b1se = P[:, _B1SE:_B1SE + 1]
            b2se = P[:, _B2SE:_B2SE + 1]
            CWT = P[:, _CWT:_CWT + 90]
            W1p = P[0:20, _W1P:_W1P + 10]
            b1col = P[0:10, _B1C:_B1C + 1]
            W2w = P[0:2, _W2W:_W2W + 10]
            b2col = P[0:2, _B2C:_B2C + 1]
            cb10 = P[0:10, _CB:_CB + 1]
            PM = P[0:2, _PM:_PM + 2]
            MASK2 = P[0:2, _MASK2:_MASK2 + 20]
            MASKO = P[0:10, _MASKO:_MASKO + 20]
            I2 = P[0:2, _I2:_I2 + 2]

            # ---------------- PSUM tiles ----------------
            junk_ps = ps.tile([2, 8], F32, tag="pE")
            v_ps = ps.tile([64, 1], F32, tag="pB")
            tbc_ps = ps.tile([64, 2], F32, tag="pC")
            bcol_ps = ps.tile([20, 1], F32, tag="pG")
            w2pm_ps = ps.tile([10, 2], F32, tag="pF")

            # ---------------- DMAs (all on SP) ----------------
            nc.sync.dma_start(out=PB[:], in_=packbf[:, :])
            nc.sync.dma_start(out=P[:], in_=packed[:, :])

            # ---------------- constants + PE warmup ----------------
            nc.vector.memset(ones64[:], 1.0)
            for _ in range(2):
                nc.tensor.matmul(
                    junk_ps[0:1, 0:1], ones64[0:1, 0:1], ones64[0:1, 0:1],
                    start=True, stop=True,
                )

            # early, DMA-ready PE work: bcol and W2pm
            bcol_i = nc.tensor.matmul(bcol_ps[:], MASKO, cb10, start=True, stop=True)
            bcolcp_i = nc.vector.tensor_copy(bcol[:], bcol_ps[:])
            w2pm_i = nc.tensor.matmul(w2pm_ps[:], W2w, PM, start=True, stop=True)
            w2pmcp_i = nc.vector.tensor_copy(W2pm[:], w2pm_ps[:])

            # ---------------- cosine stage ----------------
            # all reductions on DVE (accum_out is free there; ACT charges
            # +187ns per accumulator read). 1/sqrt(x) as sqrt(1/x): the
            # reciprocal runs BEFORE the one ACT Sqrt, so rt2 holds
            # [1/ne | 1/sab] and feeds v / D2 directly.
            # ssq_e on ACT (Square+accum); wav pair on DVE via
            # tensor_tensor + tensor_reduce (walrus here lacks
            # TensorTensorReduce: "ISA wrong length")
            nc.scalar.activation(
                Esq[:], E, ACTF.Square, accum_out=ssq2[:, 0:1]
            )
            dots_i = nc.vector.tensor_tensor(
                junkB[:], E0x2, Wab, op=ALU.mult
            )
            trd_i = nc.vector.tensor_reduce(
                dots[:], junkB[:], axis=mybir.AxisListType.X, op=ALU.add
            )
            ttr_ab = nc.vector.tensor_tensor(
                junkA[:], Wab, Wab, op=ALU.mult
            )
            tra_i = nc.vector.tensor_reduce(
                ssq2[0:2, 1:2], junkA[:], axis=mybir.AxisListType.X, op=ALU.add
            )
            recb_i = nc.vector.reciprocal(rq2[0:2, 1:2], ssq2[0:2, 1:2])
            rece_i = nc.vector.reciprocal(rq2[:, 0:1], ssq2[:, 0:1])
            nc.scalar.activation(rt2[0:2, 1:2], rq2[0:2, 1:2], ACTF.Sqrt)
            nc.scalar.activation(rt2[:, 0:1], rq2[:, 0:1], ACTF.Sqrt)
            # D2 = diag(t) = (I2 * dots) * (1/sab)
            d2_i = nc.vector.tensor_scalar(
                out=D2[:], in0=I2, scalar1=dots[:], scalar2=rt2[0:2, 1:2],
                op0=ALU.mult, op1=ALU.mult,
            )
            d2a_i = d2_i

            # ---------------- SE chain ----------------
            v_i = nc.tensor.matmul(v_ps[:], w1T, rt2[:, 0:1], start=True, stop=True)
            tbc_i = nc.tensor.matmul(
                tbc_ps[:], ones64[0:2, :], D2[:], start=True, stop=True
            )
            vcp_i = nc.vector.tensor_copy(v_sb[:], v_ps[:])
            nc.scalar.activation(
                hT[:, 0:1], tbc_ps[:, 0:1], ACTF.Tanh, bias=b1se, scale=v_sb[:]
            )
            nc.scalar.activation(
                hT[:, 1:2], tbc_ps[:, 1:2], ACTF.Tanh, bias=b1se, scale=v_sb[:]
            )
            z_ps = ps.tile([64, 2], F32, tag="pD")
            z_i = nc.tensor.matmul(z_ps[:], w2T, hT[:], start=True, stop=True)
            sT_ps = ps.tile([64, 2], F32, tag="pF")
            nc.scalar.activation(sT_ps[:, 0:1], z_ps[:, 0:1], ACTF.Sigmoid, bias=b2se)
            nc.scalar.activation(sT_ps[:, 1:2], z_ps[:, 1:2], ACTF.Sigmoid, bias=b2se)
            nc.scalar.activation(expT[:, 0:1], sT_ps[:, 0:1], ACTF.Exp)
            nc.scalar.activation(expT[:, 1:2], sT_ps[:, 1:2], ACTF.Exp)

            # stall[r,k,o*2+g]: both halves on DVE (107ns each at bf16
            # 2x rate; beats ACT's 260ns Copy)
            stall1_i = nc.vector.tensor_scalar_mul(
                stall[:, :, 0:20:2], CWT, expT[:, 0:1]
            )
            stall2_i = nc.vector.tensor_scalar_mul(
                stall[:, :, 1:20:2], CWT, expT[:, 1:2]
            )

            # softmax denominators
            cs_ps = ps.tile([2, 1], F32, tag="pB")
            cs_i = nc.tensor.matmul(
                cs_ps[:], expT[:], ones64[:, 0:1], start=True, stop=True
            )
            rs_i = nc.vector.reciprocal(rs[:], cs_ps[:])
            scol_ps = ps.tile([20, 1], F32, tag="pC")
            scol_i = nc.tensor.matmul(scol_ps[:], MASK2, rs[:], start=True, stop=True)
            scolcp_i = nc.vector.tensor_copy(scol[:], scol_ps[:])

            # ---------------- conv: 9 accumulated matmuls ----------------
            Y_ps = ps.tile([20, 120], F32, tag="pA")
            conv_is = []
            for k in range(KW):
                conv_is.append(nc.tensor.matmul(
                    Y_ps[:],
                    stall[:, k, :],
                    E[:, k:k + WOUT],
                    start=(k == 0), stop=(k == KW - 1),
                ))

            # relu(Y/colsum + b) with mean via ACT accum (the +187ns
            # accumulator read still beats a DVE round trip here)
            nc.scalar.activation(
                R[:], Y_ps[:], ACTF.Relu, bias=bcol[:], scale=scol[:],
                accum_out=msum[:],
            )

            # ---------------- fcn head ----------------
            S_ps = ps.tile([10, 1], F32, tag="pB")
            s_i = nc.tensor.matmul(S_ps[:], W1p, msum[:], start=True, stop=True)
            nc.scalar.activation(
                h2[:], S_ps[:], ACTF.Sigmoid, bias=b1col, scale=1.0 / WOUT
            )
            logit_ps = ps.tile([1, 2], F32, tag="pC")
            lg2_i = nc.tensor.matmul(logit_ps[:], b2col, PM, start=True, stop=False)
            lg1_i = nc.tensor.matmul(logit_ps[:], h2[:], W2pm[:], start=False, stop=True)
            # softmax([l0,l1]) = [sigmoid(d), sigmoid(-d)] with d = l0-l1:
            # two free-size-1 ACT ops (near-zero cost) on logit_pm[0]
            nc.scalar.activation(
                final[0:1, 0:1], logit_ps[0:1, 0:1], ACTF.Sigmoid
            )
            nc.scalar.activation(
                final[0:1, 1:2], logit_ps[0:1, 0:1], ACTF.Sigmoid, scale=-1.0
            )

            nc.sync.dma_start(out=out[:, :], in_=final[:])

            # ---- queue-order pins (scheduler-only edges, no sems) ----
            pe_order = [bcol_i, w2pm_i, v_i, tbc_i, z_i, cs_i, scol_i,
                        conv_is[0], conv_is[-1], s_i, lg2_i, lg1_i]
            for a, b in zip(pe_order[1:], pe_order[:-1]):
                add_dep_helper(a.ins, b.ins, sync=False, reason="pe order")
            dve_order = [dots_i, ttr_ab, trd_i, tra_i, recb_i, rece_i,
                         d2_i, vcp_i, bcolcp_i, w2pmcp_i, stall1_i,
                         stall2_i, rs_i, scolcp_i]
            for a, b in zip(dve_order[1:], dve_order[:-1]):
                add_dep_helper(a.ins, b.ins, sync=False, reason="dve order")

    _strip_preamble_barrier(nc)
    if split_waits:
        _split_multi_waits(nc)
    return nc


def _pack_inputs(inputs):
    import ml_dtypes
    f = {k: np.asarray(v, dtype=np.float32) for k, v in inputs.items()}
    x = f["x"].reshape(66, 128)
    eeg = x[1:65]                       # [64,128]
    pb = np.zeros((64, _FB), np.float32)
    pb[:, _E0:_E0 + 128] = eeg
    pb[0, _WAB:_WAB + 128] = x[0]
    pb[1, _WAB:_WAB + 128] = x[65]
    pb[0:2, _E0X2:_E0X2 + 128] = eeg[0]
    pk = np.zeros((64, _F), np.float32)
    pk[:, _W1T:_W1T + 64] = f["se_w1"].T
    pk[:, _W2T:_W2T + 64] = f["se_w2"].T
    pk[:, _B1SE] = f["se_b1"]
    pk[:, _B2SE] = f["se_b2"]
    # conv_w [10,1,64,9] -> [r, k, o]
    pk[:, _CWT:_CWT + 90] = np.transpose(
        f["conv_w"][:, 0, :, :], (1, 2, 0)
    ).reshape(64, 90)
    pk[0:20, _W1P:_W1P + 10] = f["fcn_w1"].T      # rows p=o*2+g
    pk[0:10, _B1C] = f["fcn_b1"]
    pk[0:2, _W2W:_W2W + 10] = f["fcn_w2"]
    pk[0:2, _B2C] = f["fcn_b2"]
    pk[0:10, _CB] = f["conv_b"]
    pk[0:2, _PM:_PM + 2] = np.array([[1.0, -1.0], [-1.0, 1.0]], np.float32)
    pk[0:2, _I2:_I2 + 2] = np.eye(2, dtype=np.float32)
    for o in range(10):
        for g in range(2):
            pk[g, _MASK2 + o * 2 + g] = 1.0
            pk[o, _MASKO + o * 2 + g] = 1.0
    return {"packbf": pb.astype(ml_dtypes.bfloat16), "packed": pk}


_NC_CACHE = None


def kernel(**inputs) -> np.ndarray:
    global _NC_CACHE
    if _NC_CACHE is None:
        _NC_CACHE = build_program()
    nc = _NC_CACHE

    in_map = _pack_inputs(inputs)
    res = run_bass_kernel_spmd(
        nc, [in_map] * N_CORES, core_ids=list(range(N_CORES))
    )
    return np.asarray(res.results[0]["out"], dtype=np.float32)


# revision 47
# speedup vs baseline: 1.1710x; 1.0293x over previous
"""Trainium2 Bass kernel for the tiny EEG CNN (nn_CNN_56745107915038).

Single-core latency-bound graph (~2.8 MFLOP), replicated SPMD on all 8
cores; core 0's output is returned. v3 design:

  - kernel() packs x-views and ALL weights into one [64, F] f32 tensor on
    the host (pure data marshaling: transposes/duplication, no math), plus
    a bf16 copy of the eeg block for the conv moving operand. The device
    program does 3 input DMAs total (pack cols 0:384 with the x-derived
    data first, the weight remainder second, xbf third), so every weight
    is on-chip by ~3.5us and no engine ever waits on a DMA slot chain.
  - e0 is packed twice so dots = one tensor_tensor_reduce against the
    [wav_a; wav_b] pair - no PE broadcast matmul.
  - se_w1.T / se_w2.T / conv_w.T / fcn_w1.T are packed pre-transposed:
    no PE transpose + PSUM->SBUF copy chains anywhere.
  - diag(t) via one two-scalar tensor_scalar (I2*dots/sab); softmax of the
    SE weights deferred: exp-scaled conv stationary (split DVE/ACT), 1/sum
    rides the Relu scale; 2-class softmax = sigmoid of PM-folded logits
    with fcn_b2's contribution as a second accumulating matmul.
  - PE p-state warmup; TileContext teardown stripped (NEFF runs once per
    PJRT dispatch).
"""

import sys

for _p in ("/opt/trn_rl_repo", "/root/.axon_site/_ro/trn_rl_repo"):
    if _p not in sys.path:
        sys.path.append(_p)

import numpy as np

from concourse import bass, mybir
from concourse import tile
from concourse.bass_utils import run_bass_kernel_spmd
from concourse.tile_rust import add_dep_helper

F32 = mybir.dt.float32
BF16 = mybir.dt.bfloat16
ALU = mybir.AluOpType
ACTF = mybir.ActivationFunctionType

N_CORES = 8
KW = 9
WOUT = 128 - KW + 1  # 120

# ---- bf16 x-pack column layout ----
_E0 = 0        # eeg [64,128]
_WAB = 128     # [wav_a; wav_b] rows 0-1, cols 128:256
_E0X2 = 256    # [e0; e0] rows 0-1, cols 256:384
_FB = 384
# ---- f32 weight-pack column layout ----
_W1T = 0       # se_w1.T [64,64]
_W2T = 64      # se_w2.T [64,64]
_B1SE = 128    # se_b1 col
_B2SE = 129    # se_b2 col
_CWT = 130     # conv_w.T [64, 9, 10] (r, k, o)
_W1P = 220     # fcn_w1.T [20,10] rows 0-19
_B1C = 230     # fcn_b1 rows 0-9
_W2W = 231     # fcn_w2 [2,10] rows 0-1
_B2C = 241     # fcn_b2 rows 0-1
_CB = 242      # conv_b rows 0-9
_PM = 243      # [[1,-1],[-1,1]] rows 0-1, cols 243:245
_MASK2 = 245   # MASK2[g, o*2+g'] = (g==g') rows 0-1, cols 245:265
_MASKO = 265   # MASKO[o, o'*2+g] = (o==o') rows 0-9, cols 265:285
_I2 = 285      # identity [2,2] rows 0-1, cols 285:287
_F = 287


def _split_multi_waits(nc):
    """Walrus in this container allows at most one sync wait per instruction.

    Tile's sem assignment freely attaches several. Hoist all but the last
    wait of each instruction onto injected same-engine NOPs placed directly
    before it -- engines execute in order, so the waits still gate it.
    """
    for fn in nc.m.functions:
        for blk in fn.blocks:
            new = []
            for inst in blk.instructions:
                si = inst.sync_info
                if si is not None and len(si.on_wait) > 1:
                    waits = sorted(
                        si.on_wait, key=lambda w: 0 if "DMA" in (w.ant_name or "") else 1
                    )
                    for j, w in enumerate(waits[:-1]):
                        new.append(
                            mybir.InstNoOp(
                                name=f"{inst.name}-swait{j}",
                                engine=inst.engine,
                                ins=[], outs=[],
                                sync_info=mybir.SyncInfo(on_wait=[w], on_update=[]),
                            )
                        )
                    inst.sync_info = mybir.SyncInfo(
                        on_wait=[waits[-1]], on_update=list(si.on_update)
                    )
                new.append(inst)
            blk.instructions = new


class _TileContext(tile.TileContext):
    """TileContext with an empty kernel tail.

    The NEFF runs once per PJRT dispatch, so semaphores never need
    resetting and the runtime's own DMA-queue quiescence covers the
    output DMA completion. Skip the drain/barrier/sem-clear sequence.
    """

    def _drain_and_barrier(self, tick_clock, wait_clock):
        popped = self.nc._tile_sem_poison_stack.pop()
        assert popped is self._sem_poison


def _strip_out_dma_sem(nc):
    """Drop the completion-sem update from the final output DMA.

    Nothing waits on it (the teardown is empty; the runtime's DMA-queue
    quiescence covers completion), and the cost model charges 900ns of
    sem propagation only when a DMA carries updates."""
    last_dma = None
    for blk in nc.m.functions[0].blocks:
        for inst in blk.instructions:
            if type(inst).__name__ == "InstDMACopy":
                last_dma = inst
    if last_dma is not None and last_dma.sync_info is not None:
        last_dma.sync_info = mybir.SyncInfo(
            on_wait=list(last_dma.sync_info.on_wait), on_update=[]
        )


def _strip_preamble_barrier(nc):
    """Drop the const-init all-engine barrier from the Bass preamble,
    and SP's RegisterMoves (they only feed SWDGE rings, which this
    program never uses) so the first input DMA issues ~300ns sooner."""
    blk0 = nc.m.functions[0].blocks[0]
    keep = []
    for i in blk0.instructions:
        tn = type(i).__name__
        if tn in ("InstDrain", "InstEventSemaphore"):
            continue
        if tn == "InstRegisterMove" and i.engine == mybir.EngineType.SP:
            continue
        keep.append(i)
    blk0.instructions = keep


def build_program(split_waits=True):
    nc = bass.Bass()

    packbf = nc.dram_tensor("packbf", [64, _FB], BF16, kind="ExternalInput")
    packed = nc.dram_tensor("packed", [64, _F], F32, kind="ExternalInput")
    out = nc.dram_tensor("out", [1, 2], F32, kind="ExternalOutput")

    with _TileContext(nc) as tc:
        with (
            tc.tile_pool(name="sb", bufs=1) as sb,
            tc.tile_pool(name="ps", bufs=1, space="PSUM") as ps,
        ):
            # ---------------- SBUF tiles ----------------
            P = sb.tile([64, _F], F32, tag="P")          # weight pack
            PB = sb.tile([64, _FB], BF16, tag="PB")      # x pack
            ones64 = sb.tile([64, 64], F32, tag="ones64")
            stall = sb.tile([64, 9, 20], BF16, tag="stall")
            junkA = sb.tile([2, 128], BF16, tag="junkA")
            junkB = sb.tile([2, 128], BF16, tag="junkB")
            Esq = sb.tile([64, 128], BF16, tag="Esq")
            ssq2 = sb.tile([64, 2], F32, tag="ssq2")     # [ssq_e | ssqab]
            rt2 = sb.tile([64, 2], F32, tag="rt2")       # sqrt of ssq2
            rq2 = sb.tile([64, 2], F32, tag="rq2")
            dots = sb.tile([2, 1], F32, tag="dots")
            D2 = sb.tile([2, 2], F32, tag="D2")
            v_sb = sb.tile([64, 1], F32, tag="v_sb")
            hT = sb.tile([64, 2], F32, tag="hT")
            expT = sb.tile([64, 2], F32, tag="expT")
            rs = sb.tile([2, 1], F32, tag="rs")
            scol = sb.tile([20, 1], F32, tag="scol")
            bcol = sb.tile([20, 1], F32, tag="bcol")
            W2pm = sb.tile([10, 2], F32, tag="W2pm")
            R = sb.tile([20, 120], F32, tag="R")
            msum = sb.tile([20, 1], F32, tag="msum")
            h2 = sb.tile([10, 1], F32, tag="h2")
            final = sb.tile([1, 2], F32, tag="final")

            # pack views
            E = PB[:, _E0:_E0 + 128]
            Wab = PB[0:2, _WAB:_WAB + 128]
            E0x2 = PB[0:2, _E0X2:_E0X2 + 128]
            w1T = P[:, _W1T:_W1T + 64]
            w2T = P[:, _W2T:_W2T + 64]
            b1se = P[:, _B1SE:_B1SE + 1]
            b2se = P[:, _B2SE:_B2SE + 1]
            CWT = P[:, _CWT:_CWT + 90]
            W1p = P[0:20, _W1P:_W1P + 10]
            b1col = P[0:10, _B1C:_B1C + 1]
            W2w = P[0:2, _W2W:_W2W + 10]
            b2col = P[0:2, _B2C:_B2C + 1]
            cb10 = P[0:10, _CB:_CB + 1]
            PM = P[0:2, _PM:_PM + 2]
            MASK2 = P[0:2, _MASK2:_MASK2 + 20]
            MASKO = P[0:10, _MASKO:_MASKO + 20]
            I2 = P[0:2, _I2:_I2 + 2]

            # ---------------- PSUM tiles ----------------
            junk_ps = ps.tile([2, 8], F32, tag="pE")
            v_ps = ps.tile([64, 1], F32, tag="pB")
            tbc_ps = ps.tile([64, 2], F32, tag="pC")
            bcol_ps = ps.tile([20, 1], F32, tag="pG")
            w2pm_ps = ps.tile([10, 2], F32, tag="pF")

            # ---------------- DMAs (all on SP) ----------------
            nc.sync.dma_start(out=PB[:], in_=packbf[:, :])
            nc.sync.dma_start(out=P[:], in_=packed[:, :])

            # ---------------- constants + PE warmup ----------------
            nc.vector.memset(ones64[:], 1.0)
            for _ in range(2):
                nc.tensor.matmul(
                    junk_ps[0:1, 0:1], ones64[0:1, 0:1], ones64[0:1, 0:1],
                    start=True, stop=True,
                )

            # early, DMA-ready PE work: bcol and W2pm
            bcol_i = nc.tensor.matmul(bcol_ps[:], MASKO, cb10, start=True, stop=True)
            bcolcp_i = nc.vector.tensor_copy(bcol[:], bcol_ps[:])
            w2pm_i = nc.tensor.matmul(w2pm_ps[:], W2w, PM, start=True, stop=True)
            w2pmcp_i = nc.vector.tensor_copy(W2pm[:], w2pm_ps[:])

            # ---------------- cosine stage ----------------
            # all reductions on DVE (accum_out is free there; ACT charges
            # +187ns per accumulator read). 1/sqrt(x) as sqrt(1/x): the
            # reciprocal runs BEFORE the one ACT Sqrt, so rt2 holds
            # [1/ne | 1/sab] and feeds v / D2 directly.
            # ssq_e on ACT (Square+accum); wav pair on DVE via
            # tensor_tensor + tensor_reduce (walrus here lacks
            # TensorTensorReduce: "ISA wrong length")
            nc.scalar.activation(
                Esq[:], E, ACTF.Square, accum_out=ssq2[:, 0:1]
            )
            dots_i = nc.vector.tensor_tensor(
                junkB[:], E0x2, Wab, op=ALU.mult
            )
            trd_i = nc.vector.tensor_reduce(
                dots[:], junkB[:], axis=mybir.AxisListType.X, op=ALU.add
            )
            ttr_ab = nc.vector.tensor_tensor(
                junkA[:], Wab, Wab, op=ALU.mult
            )
            tra_i = nc.vector.tensor_reduce(
                ssq2[0:2, 1:2], junkA[:], axis=mybir.AxisListType.X, op=ALU.add
            )
            recb_i = nc.vector.reciprocal(rq2[0:2, 1:2], ssq2[0:2, 1:2])
            rece_i = nc.vector.reciprocal(rq2[:, 0:1], ssq2[:, 0:1])
            nc.scalar.activation(rt2[0:2, 1:2], rq2[0:2, 1:2], ACTF.Sqrt)
            nc.scalar.activation(rt2[:, 0:1], rq2[:, 0:1], ACTF.Sqrt)
            # D2 = diag(t) = (I2 * dots) * (1/sab)
            d2_i = nc.vector.tensor_scalar(
                out=D2[:], in0=I2, scalar1=dots[:], scalar2=rt2[0:2, 1:2],
                op0=ALU.mult, op1=ALU.mult,
            )
            d2a_i = d2_i

            # ---------------- SE chain ----------------
            v_i = nc.tensor.matmul(v_ps[:], w1T, rt2[:, 0:1], start=True, stop=True)
            tbc_i = nc.tensor.matmul(
                tbc_ps[:], ones64[0:2, :], D2[:], start=True, stop=True
            )
            vcp_i = nc.vector.tensor_copy(v_sb[:], v_ps[:])
            nc.scalar.activation(
                hT[:, 0:1], tbc_ps[:, 0:1], ACTF.Tanh, bias=b1se, scale=v_sb[:]
            )
            nc.scalar.activation(
                hT[:, 1:2], tbc_ps[:, 1:2], ACTF.Tanh, bias=b1se, scale=v_sb[:]
            )
            z_ps = ps.tile([64, 2], F32, tag="pD")
            z_i = nc.tensor.matmul(z_ps[:], w2T, hT[:], start=True, stop=True)
            sT_ps = ps.tile([64, 2], F32, tag="pF")
            nc.scalar.activation(sT_ps[:, 0:1], z_ps[:, 0:1], ACTF.Sigmoid, bias=b2se)
            nc.scalar.activation(expT[:, 0:1], sT_ps[:, 0:1], ACTF.Exp)
            nc.scalar.activation(sT_ps[:, 1:2], z_ps[:, 1:2], ACTF.Sigmoid, bias=b2se)
            nc.scalar.activation(expT[:, 1:2], sT_ps[:, 1:2], ACTF.Exp)

            # stall[r,k,o*2+g]: both halves on DVE (107ns each at bf16
            # 2x rate; beats ACT's 260ns Copy)
            stall1_i = nc.vector.tensor_scalar_mul(
                stall[:, :, 0:20:2], CWT, expT[:, 0:1]
            )
            stall2_i = nc.vector.tensor_scalar_mul(
                stall[:, :, 1:20:2], CWT, expT[:, 1:2]
            )

            # softmax denominators
            cs_ps = ps.tile([2, 1], F32, tag="pB")
            cs_i = nc.tensor.matmul(
                cs_ps[:], expT[:], ones64[:, 0:1], start=True, stop=True
            )
            rs_i = nc.vector.reciprocal(rs[:], cs_ps[:])
            scol_ps = ps.tile([20, 1], F32, tag="pC")
            scol_i = nc.tensor.matmul(scol_ps[:], MASK2, rs[:], start=True, stop=True)
            scolcp_i = nc.vector.tensor_copy(scol[:], scol_ps[:])

            # ---------------- conv: 9 accumulated matmuls ----------------
            Y_ps = ps.tile([20, 120], F32, tag="pA")
            conv_is = []
            for k in range(KW):
                conv_is.append(nc.tensor.matmul(
                    Y_ps[:],
                    stall[:, k, :],
                    E[:, k:k + WOUT],
                    start=(k == 0), stop=(k == KW - 1),
                ))

            # relu(Y/colsum + b) with mean via ACT accum (the +187ns
            # accumulator read still beats a DVE round trip here)
            nc.scalar.activation(
                R[:], Y_ps[:], ACTF.Relu, bias=bcol[:], scale=scol[:],
                accum_out=msum[:],
            )

            # ---------------- fcn head ----------------
            S_ps = ps.tile([10, 1], F32, tag="pB")
            s_i = nc.tensor.matmul(S_ps[:], W1p, msum[:], start=True, stop=True)
            nc.scalar.activation(
                h2[:], S_ps[:], ACTF.Sigmoid, bias=b1col, scale=1.0 / WOUT
            )
            logit_ps = ps.tile([1, 2], F32, tag="pC")
            lg2_i = nc.tensor.matmul(logit_ps[:], b2col, PM, start=True, stop=False)
            lg1_i = nc.tensor.matmul(logit_ps[:], h2[:], W2pm[:], start=False, stop=True)
            # softmax([l0,l1]) = [sigmoid(d), sigmoid(-d)] with d = l0-l1:
            # two free-size-1 ACT ops (near-zero cost) on logit_pm[0]
            nc.scalar.activation(
                final[0:1, 0:1], logit_ps[0:1, 0:1], ACTF.Sigmoid
            )
            nc.scalar.activation(
                final[0:1, 1:2], logit_ps[0:1, 0:1], ACTF.Sigmoid, scale=-1.0
            )

            nc.sync.dma_start(out=out[:, :], in_=final[:])

            # ---- queue-order pins (scheduler-only edges, no sems) ----
            pe_order = [bcol_i, w2pm_i, v_i, tbc_i, z_i, cs_i, scol_i,
                        conv_is[0], conv_is[-1], s_i, lg2_i, lg1_i]
            for a, b in zip(pe_order[1:], pe_order[:-1]):
                add_dep_helper(a.ins, b.ins, sync=False, reason="pe order")
            dve_order = [dots_i, ttr_ab, trd_i, tra_i, recb_i, rece_i,
                         d2_i, vcp_i, bcolcp_i, w2pmcp_i, stall1_i,
                         stall2_i, rs_i, scolcp_i]
            for a, b in zip(dve_order[1:], dve_order[:-1]):
                add_dep_helper(a.ins, b.ins, sync=False, reason="dve order")

    _strip_preamble_barrier(nc)
    if split_waits:
        _split_multi_waits(nc)
    return nc


def _pack_inputs(inputs):
    import ml_dtypes
    f = {k: np.asarray(v, dtype=np.float32) for k, v in inputs.items()}
    x = f["x"].reshape(66, 128)
    eeg = x[1:65]                       # [64,128]
    pb = np.zeros((64, _FB), np.float32)
    pb[:, _E0:_E0 + 128] = eeg
    pb[0, _WAB:_WAB + 128] = x[0]
    pb[1, _WAB:_WAB + 128] = x[65]
    pb[0:2, _E0X2:_E0X2 + 128] = eeg[0]
    pk = np.zeros((64, _F), np.float32)
    pk[:, _W1T:_W1T + 64] = f["se_w1"].T
    pk[:, _W2T:_W2T + 64] = f["se_w2"].T
    pk[:, _B1SE] = f["se_b1"]
    pk[:, _B2SE] = f["se_b2"]
    # conv_w [10,1,64,9] -> [r, k, o]
    pk[:, _CWT:_CWT + 90] = np.transpose(
        f["conv_w"][:, 0, :, :], (1, 2, 0)
    ).reshape(64, 90)
    pk[0:20, _W1P:_W1P + 10] = f["fcn_w1"].T      # rows p=o*2+g
    pk[0:10, _B1C] = f["fcn_b1"]
    pk[0:2, _W2W:_W2W + 10] = f["fcn_w2"]
    pk[0:2, _B2C] = f["fcn_b2"]
    pk[0:10, _CB] = f["conv_b"]
    pk[0:2, _PM:_PM + 2] = np.array([[1.0, -1.0], [-1.0, 1.0]], np.float32)
    pk[0:2, _I2:_I2 + 2] = np.eye(2, dtype=np.float32)
    for o in range(10):
        for g in range(2):
            pk[g, _MASK2 + o * 2 + g] = 1.0
            pk[o, _MASKO + o * 2 + g] = 1.0
    return {"packbf": pb.astype(ml_dtypes.bfloat16), "packed": pk}


_NC_CACHE = None


def kernel(**inputs) -> np.ndarray:
    global _NC_CACHE
    if _NC_CACHE is None:
        _NC_CACHE = build_program()
    nc = _NC_CACHE

    in_map = _pack_inputs(inputs)
    res = run_bass_kernel_spmd(
        nc, [in_map] * N_CORES, core_ids=list(range(N_CORES))
    )
    return np.asarray(res.results[0]["out"], dtype=np.float32)


# revision 49
# speedup vs baseline: 1.1769x; 1.0050x over previous
"""Trainium2 Bass kernel for the tiny EEG CNN (nn_CNN_56745107915038).

Single-core latency-bound graph (~2.8 MFLOP), replicated SPMD on all 8
cores; core 0's output is returned. v3 design:

  - kernel() packs x-views and ALL weights into one [64, F] f32 tensor on
    the host (pure data marshaling: transposes/duplication, no math), plus
    a bf16 copy of the eeg block for the conv moving operand. The device
    program does 3 input DMAs total (pack cols 0:384 with the x-derived
    data first, the weight remainder second, xbf third), so every weight
    is on-chip by ~3.5us and no engine ever waits on a DMA slot chain.
  - e0 is packed twice so dots = one tensor_tensor_reduce against the
    [wav_a; wav_b] pair - no PE broadcast matmul.
  - se_w1.T / se_w2.T / conv_w.T / fcn_w1.T are packed pre-transposed:
    no PE transpose + PSUM->SBUF copy chains anywhere.
  - diag(t) via one two-scalar tensor_scalar (I2*dots/sab); softmax of the
    SE weights deferred: exp-scaled conv stationary (split DVE/ACT), 1/sum
    rides the Relu scale; 2-class softmax = sigmoid of PM-folded logits
    with fcn_b2's contribution as a second accumulating matmul.
  - PE p-state warmup; TileContext teardown stripped (NEFF runs once per
    PJRT dispatch).
"""

import sys

for _p in ("/opt/trn_rl_repo", "/root/.axon_site/_ro/trn_rl_repo"):
    if _p not in sys.path:
        sys.path.append(_p)

import numpy as np

from concourse import bass, mybir
from concourse import tile
from concourse.bass_utils import run_bass_kernel_spmd
from concourse.tile_rust import add_dep_helper

F32 = mybir.dt.float32
BF16 = mybir.dt.bfloat16
ALU = mybir.AluOpType
ACTF = mybir.ActivationFunctionType

N_CORES = 8
KW = 9
WOUT = 128 - KW + 1  # 120

# ---- bf16 x-pack column layout ----
_E0 = 0        # eeg [64,128]
_WAB = 128     # [wav_a; wav_b] rows 0-1, cols 128:256
_E0X2 = 256    # [e0; e0] rows 0-1, cols 256:384
_FB = 384
# ---- f32 weight-pack column layout ----
_W1T = 0       # se_w1.T [64,64]
_W2T = 64      # se_w2.T [64,64]
_B1SE = 128    # se_b1 col
_B2SE = 129    # se_b2 col
_CWT = 130     # conv_w.T [64, 9, 10] (r, k, o)
_W1P = 220     # fcn_w1.T [20,10] rows 0-19
_B1C = 230     # fcn_b1 rows 0-9
_W2W = 231     # fcn_w2 [2,10] rows 0-1
_B2C = 241     # fcn_b2 rows 0-1
_CB = 242      # conv_b rows 0-9
_PM = 243      # [[1,-1],[-1,1]] rows 0-1, cols 243:245
_MASK2 = 245   # MASK2[g, o*2+g'] = (g==g') rows 0-1, cols 245:265
_MASKO = 265   # MASKO[o, o'*2+g] = (o==o') rows 0-9, cols 265:285
_I2 = 285      # identity [2,2] rows 0-1, cols 285:287
_F = 287


def _split_multi_waits(nc):
    """Walrus in this container allows at most one sync wait per instruction.

    Tile's sem assignment freely attaches several. Hoist all but the last
    wait of each instruction onto injected same-engine NOPs placed directly
    before it -- engines execute in order, so the waits still gate it.
    """
    for fn in nc.m.functions:
        for blk in fn.blocks:
            new = []
            for inst in blk.instructions:
                si = inst.sync_info
                if si is not None and len(si.on_wait) > 1:
                    waits = sorted(
                        si.on_wait, key=lambda w: 0 if "DMA" in (w.ant_name or "") else 1
                    )
                    for j, w in enumerate(waits[:-1]):
                        new.append(
                            mybir.InstNoOp(
                                name=f"{inst.name}-swait{j}",
                                engine=inst.engine,
                                ins=[], outs=[],
                                sync_info=mybir.SyncInfo(on_wait=[w], on_update=[]),
                            )
                        )
                    inst.sync_info = mybir.SyncInfo(
                        on_wait=[waits[-1]], on_update=list(si.on_update)
                    )
                new.append(inst)
            blk.instructions = new


class _TileContext(tile.TileContext):
    """TileContext with an empty kernel tail.

    The NEFF runs once per PJRT dispatch, so semaphores never need
    resetting and the runtime's own DMA-queue quiescence covers the
    output DMA completion. Skip the drain/barrier/sem-clear sequence.
    """

    def _drain_and_barrier(self, tick_clock, wait_clock):
        popped = self.nc._tile_sem_poison_stack.pop()
        assert popped is self._sem_poison


def _strip_out_dma_sem(nc):
    """Drop the completion-sem update from the final output DMA.

    Nothing waits on it (the teardown is empty; the runtime's DMA-queue
    quiescence covers completion), and the cost model charges 900ns of
    sem propagation only when a DMA carries updates."""
    last_dma = None
    for blk in nc.m.functions[0].blocks:
        for inst in blk.instructions:
            if type(inst).__name__ == "InstDMACopy":
                last_dma = inst
    if last_dma is not None and last_dma.sync_info is not None:
        last_dma.sync_info = mybir.SyncInfo(
            on_wait=list(last_dma.sync_info.on_wait), on_update=[]
        )


def _strip_preamble_barrier(nc):
    """Drop the const-init all-engine barrier from the Bass preamble,
    and SP's RegisterMoves (they only feed SWDGE rings, which this
    program never uses) so the first input DMA issues ~300ns sooner."""
    blk0 = nc.m.functions[0].blocks[0]
    keep = []
    for i in blk0.instructions:
        tn = type(i).__name__
        if tn in ("InstDrain", "InstEventSemaphore"):
            continue
        if tn == "InstRegisterMove" and i.engine == mybir.EngineType.SP:
            continue
        keep.append(i)
    blk0.instructions = keep


def build_program(split_waits=True):
    nc = bass.Bass()

    packbf = nc.dram_tensor("packbf", [64, _FB], BF16, kind="ExternalInput")
    packed = nc.dram_tensor("packed", [64, _F], F32, kind="ExternalInput")
    out = nc.dram_tensor("out", [1, 2], F32, kind="ExternalOutput")

    with _TileContext(nc) as tc:
        with (
            tc.tile_pool(name="sb", bufs=1) as sb,
            tc.tile_pool(name="ps", bufs=1, space="PSUM") as ps,
        ):
            # ---------------- SBUF tiles ----------------
            P = sb.tile([64, _F], F32, tag="P")          # weight pack
            PB = sb.tile([64, _FB], BF16, tag="PB")      # x pack
            ones64 = sb.tile([64, 64], F32, tag="ones64")
            stall = sb.tile([64, 9, 20], BF16, tag="stall")
            junkA = sb.tile([2, 128], BF16, tag="junkA")
            junkB = sb.tile([2, 128], BF16, tag="junkB")
            Esq = sb.tile([64, 128], BF16, tag="Esq")
            ssq2 = sb.tile([64, 2], F32, tag="ssq2")     # [ssq_e | ssqab]
            rt2 = sb.tile([64, 2], F32, tag="rt2")       # sqrt of ssq2
            rq2 = sb.tile([64, 2], F32, tag="rq2")
            dots = sb.tile([2, 1], F32, tag="dots")
            D2 = sb.tile([2, 2], F32, tag="D2")
            v_sb = sb.tile([64, 1], F32, tag="v_sb")
            hT = sb.tile([64, 2], F32, tag="hT")
            expT = sb.tile([64, 2], F32, tag="expT")
            rs = sb.tile([2, 1], F32, tag="rs")
            scol = sb.tile([20, 1], F32, tag="scol")
            bcol = sb.tile([20, 1], F32, tag="bcol")
            W2pm = sb.tile([10, 2], F32, tag="W2pm")

            msum = sb.tile([20, 1], F32, tag="msum")
            h2 = sb.tile([10, 1], F32, tag="h2")
            final = sb.tile([1, 2], F32, tag="final")

            # pack views
            E = PB[:, _E0:_E0 + 128]
            Wab = PB[0:2, _WAB:_WAB + 128]
            E0x2 = PB[0:2, _E0X2:_E0X2 + 128]
            w1T = P[:, _W1T:_W1T + 64]
            w2T = P[:, _W2T:_W2T + 64]
            b1se = P[:, _B1SE:_B1SE + 1]
            b2se = P[:, _B2SE:_B2SE + 1]
            CWT = P[:, _CWT:_CWT + 90]
            W1p = P[0:20, _W1P:_W1P + 10]
            b1col = P[0:10, _B1C:_B1C + 1]
            W2w = P[0:2, _W2W:_W2W + 10]
            b2col = P[0:2, _B2C:_B2C + 1]
            cb10 = P[0:10, _CB:_CB + 1]
            PM = P[0:2, _PM:_PM + 2]
            MASK2 = P[0:2, _MASK2:_MASK2 + 20]
            MASKO = P[0:10, _MASKO:_MASKO + 20]
            I2 = P[0:2, _I2:_I2 + 2]

            # ---------------- PSUM tiles ----------------
            junk_ps = ps.tile([2, 8], F32, tag="pE")
            v_ps = ps.tile([64, 1], F32, tag="pB")
            tbc_ps = ps.tile([64, 2], F32, tag="pC")
            bcol_ps = ps.tile([20, 1], F32, tag="pG")
            w2pm_ps = ps.tile([10, 2], F32, tag="pF")

            # ---------------- DMAs (all on SP) ----------------
            nc.sync.dma_start(out=PB[:], in_=packbf[:, :])
            nc.sync.dma_start(out=P[:], in_=packed[:, :])

            # ---------------- constants + PE warmup ----------------
            nc.vector.memset(ones64[:], 1.0)
            for _ in range(2):
                nc.tensor.matmul(
                    junk_ps[0:1, 0:1], ones64[0:1, 0:1], ones64[0:1, 0:1],
                    start=True, stop=True,
                )

            # early, DMA-ready PE work: bcol and W2pm
            bcol_i = nc.tensor.matmul(bcol_ps[:], MASKO, cb10, start=True, stop=True)
            bcolcp_i = nc.vector.tensor_copy(bcol[:], bcol_ps[:])
            w2pm_i = nc.tensor.matmul(w2pm_ps[:], W2w, PM, start=True, stop=True)
            w2pmcp_i = nc.vector.tensor_copy(W2pm[:], w2pm_ps[:])

            # ---------------- cosine stage ----------------
            # all reductions on DVE (accum_out is free there; ACT charges
            # +187ns per accumulator read). 1/sqrt(x) as sqrt(1/x): the
            # reciprocal runs BEFORE the one ACT Sqrt, so rt2 holds
            # [1/ne | 1/sab] and feeds v / D2 directly.
            # ssq_e on ACT (Square+accum); wav pair on DVE via
            # tensor_tensor + tensor_reduce (walrus here lacks
            # TensorTensorReduce: "ISA wrong length")
            nc.scalar.activation(
                Esq[:], E, ACTF.Square, accum_out=ssq2[:, 0:1]
            )
            dots_i = nc.vector.tensor_tensor(
                junkB[:], E0x2, Wab, op=ALU.mult
            )
            trd_i = nc.vector.tensor_reduce(
                dots[:], junkB[:], axis=mybir.AxisListType.X, op=ALU.add
            )
            ttr_ab = nc.vector.tensor_tensor(
                junkA[:], Wab, Wab, op=ALU.mult
            )
            tra_i = nc.vector.tensor_reduce(
                ssq2[0:2, 1:2], junkA[:], axis=mybir.AxisListType.X, op=ALU.add
            )
            recb_i = nc.vector.reciprocal(rq2[0:2, 1:2], ssq2[0:2, 1:2])
            rece_i = nc.vector.reciprocal(rq2[:, 0:1], ssq2[:, 0:1])
            nc.scalar.activation(rt2[0:2, 1:2], rq2[0:2, 1:2], ACTF.Sqrt)
            nc.scalar.activation(rt2[:, 0:1], rq2[:, 0:1], ACTF.Sqrt)
            # D2 = diag(t) = (I2 * dots) * (1/sab)
            d2_i = nc.vector.tensor_scalar(
                out=D2[:], in0=I2, scalar1=dots[:], scalar2=rt2[0:2, 1:2],
                op0=ALU.mult, op1=ALU.mult,
            )
            d2a_i = d2_i

            # ---------------- SE chain ----------------
            v_i = nc.tensor.matmul(v_ps[:], w1T, rt2[:, 0:1], start=True, stop=True)
            tbc_i = nc.tensor.matmul(
                tbc_ps[:], ones64[0:2, :], D2[:], start=True, stop=True
            )
            vcp_i = nc.vector.tensor_copy(v_sb[:], v_ps[:])
            nc.scalar.activation(
                hT[:, 0:1], tbc_ps[:, 0:1], ACTF.Tanh, bias=b1se, scale=v_sb[:]
            )
            nc.scalar.activation(
                hT[:, 1:2], tbc_ps[:, 1:2], ACTF.Tanh, bias=b1se, scale=v_sb[:]
            )
            z_ps = ps.tile([64, 2], F32, tag="pD")
            z_i = nc.tensor.matmul(z_ps[:], w2T, hT[:], start=True, stop=True)
            sT_ps = ps.tile([64, 2], F32, tag="pF")
            nc.scalar.activation(sT_ps[:, 0:1], z_ps[:, 0:1], ACTF.Sigmoid, bias=b2se)
            nc.scalar.activation(expT[:, 0:1], sT_ps[:, 0:1], ACTF.Exp)
            nc.scalar.activation(sT_ps[:, 1:2], z_ps[:, 1:2], ACTF.Sigmoid, bias=b2se)
            nc.scalar.activation(expT[:, 1:2], sT_ps[:, 1:2], ACTF.Exp)

            # stall[r,k,o*2+g]: both halves on DVE (107ns each at bf16
            # 2x rate; beats ACT's 260ns Copy)
            stall1_i = nc.vector.tensor_scalar_mul(
                stall[:, :, 0:20:2], CWT, expT[:, 0:1]
            )
            stall2_i = nc.vector.tensor_scalar_mul(
                stall[:, :, 1:20:2], CWT, expT[:, 1:2]
            )

            # softmax denominators
            cs_ps = ps.tile([2, 1], F32, tag="pB")
            cs_i = nc.tensor.matmul(
                cs_ps[:], expT[:], ones64[:, 0:1], start=True, stop=True
            )
            rs_i = nc.vector.reciprocal(rs[:], cs_ps[:])
            scol_ps = ps.tile([20, 1], F32, tag="pC")
            scol_i = nc.tensor.matmul(scol_ps[:], MASK2, rs[:], start=True, stop=True)
            scolcp_i = nc.vector.tensor_copy(scol[:], scol_ps[:])

            # ---------------- conv: 9 accumulated matmuls ----------------
            Y_ps = ps.tile([20, 120], F32, tag="pA")
            conv_is = []
            for k in range(KW):
                conv_is.append(nc.tensor.matmul(
                    Y_ps[:],
                    stall[:, k, :],
                    E[:, k:k + WOUT],
                    start=(k == 0), stop=(k == KW - 1),
                ))

            # relu(Y/colsum + b) with mean via ACT accum; R lives in PSUM
            # (write-only scratch - both operands PSUM = faster ACT access)
            R_ps = ps.tile([20, 120], F32, tag="pH")
            nc.scalar.activation(
                R_ps[:], Y_ps[:], ACTF.Relu, bias=bcol[:], scale=scol[:],
                accum_out=msum[:],
            )

            # ---------------- fcn head ----------------
            S_ps = ps.tile([10, 1], F32, tag="pB")
            s_i = nc.tensor.matmul(S_ps[:], W1p, msum[:], start=True, stop=True)
            nc.scalar.activation(
                h2[:], S_ps[:], ACTF.Sigmoid, bias=b1col, scale=1.0 / WOUT
            )
            logit_ps = ps.tile([1, 2], F32, tag="pC")
            lg2_i = nc.tensor.matmul(logit_ps[:], b2col, PM, start=True, stop=False)
            lg1_i = nc.tensor.matmul(logit_ps[:], h2[:], W2pm[:], start=False, stop=True)
            # softmax([l0,l1]) = [sigmoid(d), sigmoid(-d)] with d = l0-l1:
            # two free-size-1 ACT ops (near-zero cost) on logit_pm[0]
            nc.scalar.activation(
                final[0:1, 0:1], logit_ps[0:1, 0:1], ACTF.Sigmoid
            )
            nc.scalar.activation(
                final[0:1, 1:2], logit_ps[0:1, 0:1], ACTF.Sigmoid, scale=-1.0
            )

            nc.sync.dma_start(out=out[:, :], in_=final[:])

            # ---- queue-order pins (scheduler-only edges, no sems) ----
            pe_order = [bcol_i, w2pm_i, v_i, tbc_i, z_i, cs_i, scol_i,
                        conv_is[0], conv_is[-1], s_i, lg2_i, lg1_i]
            for a, b in zip(pe_order[1:], pe_order[:-1]):
                add_dep_helper(a.ins, b.ins, sync=False, reason="pe order")
            dve_order = [dots_i, ttr_ab, trd_i, tra_i, recb_i, rece_i,
                         d2_i, vcp_i, bcolcp_i, w2pmcp_i, stall1_i,
                         stall2_i, rs_i, scolcp_i]
            for a, b in zip(dve_order[1:], dve_order[:-1]):
                add_dep_helper(a.ins, b.ins, sync=False, reason="dve order")

    _strip_preamble_barrier(nc)
    if split_waits:
        _split_multi_waits(nc)
    return nc


def _pack_inputs(inputs):
    import ml_dtypes
    f = {k: np.asarray(v, dtype=np.float32) for k, v in inputs.items()}
    x = f["x"].reshape(66, 128)
    eeg = x[1:65]                       # [64,128]
    pb = np.zeros((64, _FB), np.float32)
    pb[:, _E0:_E0 + 128] = eeg
    pb[0, _WAB:_WAB + 128] = x[0]
    pb[1, _WAB:_WAB + 128] = x[65]
    pb[0:2, _E0X2:_E0X2 + 128] = eeg[0]
    pk = np.zeros((64, _F), np.float32)
    pk[:, _W1T:_W1T + 64] = f["se_w1"].T
    pk[:, _W2T:_W2T + 64] = f["se_w2"].T
    pk[:, _B1SE] = f["se_b1"]
    pk[:, _B2SE] = f["se_b2"]
    # conv_w [10,1,64,9] -> [r, k, o]
    pk[:, _CWT:_CWT + 90] = np.transpose(
        f["conv_w"][:, 0, :, :], (1, 2, 0)
    ).reshape(64, 90)
    pk[0:20, _W1P:_W1P + 10] = f["fcn_w1"].T      # rows p=o*2+g
    pk[0:10, _B1C] = f["fcn_b1"]
    pk[0:2, _W2W:_W2W + 10] = f["fcn_w2"]
    pk[0:2, _B2C] = f["fcn_b2"]
    pk[0:10, _CB] = f["conv_b"]
    pk[0:2, _PM:_PM + 2] = np.array([[1.0, -1.0], [-1.0, 1.0]], np.float32)
    pk[0:2, _I2:_I2 + 2] = np.eye(2, dtype=np.float32)
    for o in range(10):
        for g in range(2):
            pk[g, _MASK2 + o * 2 + g] = 1.0
            pk[o, _MASKO + o * 2 + g] = 1.0
    return {"packbf": pb.astype(ml_dtypes.bfloat16), "packed": pk}


_NC_CACHE = None


def kernel(**inputs) -> np.ndarray:
    global _NC_CACHE
    if _NC_CACHE is None:
        _NC_CACHE = build_program()
    nc = _NC_CACHE

    in_map = _pack_inputs(inputs)
    res = run_bass_kernel_spmd(
        nc, [in_map] * N_CORES, core_ids=list(range(N_CORES))
    )
    return np.asarray(res.results[0]["out"], dtype=np.float32)
